# revision 44
# baseline (speedup 1.0000x reference)
"""Trainium2 Bass kernel for nn_CalibrationError (ECE/MCE over softmax confidences).

Contract: kernel(logits[N,C] f32, labels[N] int64) -> (ece, mce) f32 scalars,
matching reference.py. Internally shards rows across 8 NeuronCores, computes a
15-bin cumulative (sum_conf, sum_acc, count) histogram on-device per core, and
finishes the tiny ECE/MCE arithmetic on host.

Design (fp16 end-to-end, 4-engine balance, pair-batched back half):
  - Host casts logits to fp16 (halves HBM traffic to ~50MB/core) and gathers
    xlab[i] = x16[i, label[i]] so accuracy is just (xlab == rowmax(x)) -- no
    packed-argmax pass over the full data.
  - Act engine does the one unavoidable full pass, e = exp(x) fp16 (~0.87
    ns/elem), split into two column halves so PE row-sums start early; plus a
    tiny exp(mx) per tile pair (same Exp table -> single ACT_TABLE_LOAD).
  - DVE row max: tensor_tensor max tree 100->50->25->13->7 in fp16 2x mode
    (tensor_reduce has no fast mode, so only the last 7-wide step uses it;
    overlapping slices make odd widths free since max tolerates duplicates).
  - PE row sums: identity matmuls accumulate 4-column partials in PSUM (fp16
    moving = 1 cyc/row).
  - The whole post-sum chain (s fold, clamp, reciprocal_approx_fast,
    conf/acc/ones, 16-bin compare, histogram matmuls) is batched over TILE
    PAIRS: double-width tensors halve the ~300ns/instr DVE fixed costs.
  - vals = [conf, conf, acc, ones]: the duplicated conf gives the 16-bin
    threshold compare a stride-1 fp16 last dim (viewed [p,rr,8,2]) -> DVE 2x.
  - Histogram: 8-row-batched PE matmuls (stationary = contiguous 128-wide ge
    slice -- walrus allows exactly one free dim) into one persistent [128,32]
    PSUM accumulator across all tiles; host sums the diagonal [16,4] blocks
    (bin 15 is a dummy with threshold 2.0 so blocks tile the partitions).
  - Per-pair work is software-pipelined one pair deep (front: DMA/exp/tree/
    row-sums; back: s-chain/ge/histogram) so the in-order DVE/Act/PE queues
    never stall on their own pair's cross-engine chain; x DMA is prefetched
    a tile ahead and all xlab rows arrive in one upfront DMA.
  - Pad rows use x = -1000: exp underflows to exactly 0, so conf = 0 and the
    strict conf > 0/15 compare excludes them from every bin.

Self-contained: hardcodes shapes/sharding; only imports the concourse toolchain.
"""

import sys

if "/opt/trn_rl_repo" not in sys.path:
    sys.path.insert(0, "/opt/trn_rl_repo")

import numpy as np

import concourse.bass as bass
import concourse.bacc as bacc
import concourse.mybir as mybir
from concourse.tile import TileContext
from contextlib import ExitStack

# ---------------------------------------------------------------- constants
P = 128          # SBUF partitions
C = 100          # classes
R = 104          # rows per partition per tile
T = 19           # tiles per core: 9 pairs + one single-width tail tile
NCORES = 8
NBINS = 15
NB2 = 16         # bins padded to 16 (bin 15 is a dummy with threshold 2.0) so
                 # the [16,K] histogram blocks tile the 128 PSUM partitions
KV = 4           # vals lanes: [conf, conf-dup, acc, ones]; the duplicated
                 # conf gives the ge compare a stride-1 last dim (DVE 2x mode)
G = 4            # columns per PE row-sum matmul (C = 25 * G exactly)
HJ = 8           # rows per histogram matmul
ROWS_PER_CORE = P * R * T          # 252_928 (incl. padding)
REAL_ROWS_PER_CORE = 2_000_000 // NCORES  # 250_000
PAD = -1000.0    # pad logit: exp() underflows to exactly 0.0

f16 = mybir.dt.float16
f32 = mybir.dt.float32
i16 = mybir.dt.int16
Alu = mybir.AluOpType
Act = mybir.ActivationFunctionType


def build_nc(p=P, c=C, r=R, t=T):
    """Build the per-core Bass module (SPMD: same program on all cores)."""
    nc = bacc.Bacc()
    r2 = 2 * r

    x = nc.declare_dram_parameter("x", [t * p * r, c], f16, isOutput=False)
    xlab = nc.declare_dram_parameter("xlab", [t, p, r], f16, isOutput=False)
    ident = nc.declare_dram_parameter("ident", [p, p], f16, isOutput=False)
    thr = nc.declare_dram_parameter("thr", [p, r2 * NB2], f16, isOutput=False)
    out = nc.declare_dram_parameter("out", [NB2 * HJ, KV * HJ], f32, isOutput=True)

    xv = x[:, :].rearrange("(t p r) c -> t p (r c)", t=t, p=p, r=r)

    with TileContext(nc) as tc, ExitStack() as ctx:
        consts = ctx.enter_context(tc.tile_pool(name="consts", bufs=1))
        xpool = ctx.enter_context(tc.tile_pool(name="xpool", bufs=3))
        work = ctx.enter_context(tc.tile_pool(name="work", bufs=2))
        pairp = ctx.enter_context(tc.tile_pool(name="pairp", bufs=2))
        psum = ctx.enter_context(tc.tile_pool(name="psum", bufs=4, space="PSUM"))
        psacc = ctx.enter_context(tc.tile_pool(name="psacc", bufs=1, space="PSUM"))

        ident_t = consts.tile([p, p], f16, tag="ident_t")
        nc.sync.dma_start(out=ident_t[:], in_=ident[:, :])
        xlab_t = consts.tile([p, t * r], f16, tag="xlab_t")
        thr_full = consts.tile([p, r2 * NB2], f16, tag="thr_full")

        def emit_const_dmas():
            # emitted after the first x tiles: only needed by the first back
            nc.sync.dma_start(
                out=xlab_t[:].rearrange("p (t r) -> p t r", t=t),
                in_=xlab[:, :, :].rearrange("t p r -> p t r"),
            )
            # thr_full[p, rr, b] = b / 15 (fp16), constant across rr (b
            # fastest so histogram stationary slices are one contiguous dim)
            nc.sync.dma_start(out=thr_full[:], in_=thr[:, :])
        # histogram PSUM accumulator, one group across ALL tiles' matmuls
        ph = psacc.tile([NB2 * HJ, KV * HJ], f32, tag="ph")

        # Engine warmups: absorb the const-tile DMA waits on throwaway ops so
        # first-iteration instructions carry few sync waits (walrus limits
        # the wait-command count per instruction).
        warm = psacc.tile([p, 1], f32, tag="warm")
        nc.tensor.matmul(
            warm[:], lhsT=ident_t[:], rhs=ident_t[:, 0:1], start=True, stop=True
        )
        scr_v = consts.tile([p, 1], f16, tag="scr_v")
        nc.vector.tensor_copy(out=scr_v[:], in_=ident_t[:, 0:1])
        scr_g = consts.tile([p, 1], f16, tag="scr_g")
        nc.gpsimd.tensor_tensor(
            out=scr_g[:], in0=ident_t[:, 0:1], in1=scr_v[:], op=Alu.add
        )
        scr_a = consts.tile([p, 1], f16, tag="scr_a")
        nc.scalar.activation(out=scr_a[:], in_=ident_t[:, 0:1], func=Act.Exp)

        def emit_dma(it, chunks=2):
            xt = xpool.tile([p, r * c], f16, tag="xt")
            h = (r // 2) * c
            if chunks == 4:
                q = h // 2
                nc.sync.dma_start(out=xt[:, 0:q], in_=xv[it][:, 0:q])
                nc.sync.dma_start(out=xt[:, q:h], in_=xv[it][:, q:h])
            else:
                nc.sync.dma_start(out=xt[:, 0:h], in_=xv[it][:, 0:h])
            nc.sync.dma_start(out=xt[:, h:], in_=xv[it][:, h:])
            return xt

        def new_pair(j):
            mx2 = pairp.tile([p, r2], f16, tag="mx2")
            return {"j": j, "mx2": mx2, "pss": []}

        def emit_front(it, xt, pair, half):
            """exp + max tree + PE row-sum matmuls for tile `it` (one half of
            a pair; the row max lands in its half of the pair's mx2)."""
            x3 = xt[:].rearrange("p (r c) -> p r c", r=r)

            # e = exp(x), fp16, one instruction per tile: a second instr
            # costs ~370ns of Act SBUF-access setup and Act is the wall
            et = work.tile([p, r * c], f16, tag="et")
            e3 = et[:].rearrange("p (r c) -> p r c", r=r)
            nc.scalar.activation(out=et[:], in_=xt[:], func=Act.Exp)

            # row max over classes: tensor_tensor tree (fp16 2x mode); odd
            # widths via overlapping slices (duplicates are free for max)
            m50 = work.tile([p, r * 50], f16, tag="m50")
            m50v = m50[:].rearrange("p (r c) -> p r c", r=r)
            nc.vector.tensor_tensor(
                out=m50v, in0=x3[:, :, 0:50], in1=x3[:, :, 50:100], op=Alu.max
            )
            m25 = work.tile([p, r * 25], f16, tag="m25")
            m25v = m25[:].rearrange("p (r c) -> p r c", r=r)
            nc.vector.tensor_tensor(
                out=m25v, in0=m50v[:, :, 0:25], in1=m50v[:, :, 25:50], op=Alu.max
            )
            m13 = work.tile([p, r * 13], f16, tag="m13")
            m13v = m13[:].rearrange("p (r c) -> p r c", r=r)
            nc.vector.tensor_tensor(
                out=m13v, in0=m25v[:, :, 0:13], in1=m25v[:, :, 12:25], op=Alu.max
            )
            m7 = work.tile([p, r * 7], f16, tag="m7")
            m7v = m7[:].rearrange("p (r c) -> p r c", r=r)
            nc.vector.tensor_tensor(
                out=m7v, in0=m13v[:, :, 0:7], in1=m13v[:, :, 6:13], op=Alu.max
            )
            nc.vector.tensor_reduce(
                out=pair["mx2"][:, half * r : (half + 1) * r],
                in_=m7v,
                axis=mybir.AxisListType.X,
                op=Alu.max,
            )

            # row sums on PE: 25 identity matmuls of G=4 columns accumulate
            # s-partials in PSUM; DVE folds the partials in the pair's back.
            pss = psum.tile([p, r * G], f32, tag="pss")
            for k in range(c // G):
                nc.tensor.matmul(
                    pss[:],
                    lhsT=ident_t[:],
                    rhs=e3[:, :, k * G : (k + 1) * G],
                    start=(k == 0),
                    stop=(k == c // G - 1),
                )
            pair["pss"].append(pss)

        def emit_back_act(pair, nt=2):
            """exp(mx) for the pair -- emitted before the next tiles' big exps
            so it doesn't queue behind them on the Act engine."""
            w = nt * r
            num2 = pairp.tile([p, r2], f32, tag="num2")
            nc.scalar.activation(
                out=num2[:, 0:w], in_=pair["mx2"][:, 0:w], func=Act.Exp
            )
            return num2

        def emit_back(pair, num2, nt=2, last=False):
            """Double-width s-chain + vals + ge + histogram matmuls for a tile
            pair (emitted one pair late, so the in-order DVE/PE queues never
            stall on this pair's own cross-engine chain). nt=1 handles the
            de-paired tail tiles at single width."""
            j = pair["j"]
            w = nt * r
            s2 = pairp.tile([p, r2], f32, tag="s2")
            for h in range(nt):
                nc.vector.tensor_reduce(
                    out=s2[:, h * r : (h + 1) * r],
                    in_=pair["pss"][h][:].rearrange("p (r g) -> p r g", r=r),
                    axis=mybir.AxisListType.X,
                    op=Alu.add,
                )
            # conf = exp(mx) * 1/max(s, eps); pad rows have exp(mx) == 0
            nc.vector.tensor_scalar_max(s2[:, 0:w], s2[:, 0:w], 1e-30)
            rs2 = pairp.tile([p, r2], f32, tag="rs2")
            nc.vector.reciprocal_approx_fast(out=rs2[:, 0:w], in_=s2[:, 0:w])

            # vals = [conf, conf, acc, ones] laid out [p, rr, 4] (conf twice,
            # adjacent, so ge's in0 has a stride-1 last dim -> DVE 2x mode)
            vals2 = pairp.tile([p, r2 * KV], f16, tag="vals2")
            v4 = vals2[:].rearrange("p (r k) -> p r k", k=KV)[:, 0:w, :]
            nc.vector.tensor_tensor(
                out=v4[:, :, 0:2],
                in0=num2[:, 0:w].rearrange("p (r one) -> p r one", one=1)
                .broadcast_to((p, w, 2)),
                in1=rs2[:, 0:w].rearrange("p (r one) -> p r one", one=1)
                .broadcast_to((p, w, 2)),
                op=Alu.mult,
            )
            nc.vector.tensor_tensor(
                out=v4[:, :, 2],
                in0=xlab_t[:, j * r : j * r + w],
                in1=pair["mx2"][:, 0:w],
                op=Alu.is_equal,
            )
            nc.gpsimd.memset(v4[:, :, 3], 1.0)

            # ge[p, rr, b] = conf16 > b/15 (strict: pad rows have conf == 0).
            # View both sides as [p, rr, 8, 2]: conf pair (stride 1) vs bin
            # pairs (2j, 2j+1) -> every operand 2-byte with stride-1 last dim.
            ge2 = pairp.tile([p, r2 * NB2], f16, tag="ge2")
            g4 = ge2[:].rearrange("p (r j k) -> p r j k", j=NB2 // 2, k=2)[
                :, 0:w, :, :
            ]
            t4 = thr_full[:].rearrange("p (r j k) -> p r j k", j=NB2 // 2, k=2)[
                :, 0:w, :, :
            ]
            c4 = (
                v4[:, :, 0:2]
                .rearrange("p r (j k) -> p r j k", j=1)
                .broadcast_to((p, w, NB2 // 2, 2))
            )
            nc.vector.tensor_tensor(out=g4, in0=c4, in1=t4, op=Alu.is_gt)

            # histogram: 28 matmuls of 8 rows each into the persistent
            # [128,32] PSUM accumulator; diagonal [16,4] blocks hold the real
            # sums and are block-summed on host after one DMA at kernel end.
            # Stationary = 8-row ge slice (one contiguous 128-elem free dim,
            # walrus requires exactly one); moving = vals slice [8, 4].
            vrb = vals2[:].rearrange("p (r k) -> p r k", k=KV)
            for rb in range(w // HJ):
                nc.tensor.matmul(
                    ph[:],
                    lhsT=ge2[:, rb * HJ * NB2 : (rb + 1) * HJ * NB2],
                    rhs=vrb[:, rb * HJ : (rb + 1) * HJ, :],
                    start=(j == 0 and rb == 0),
                    stop=(last and rb == w // HJ - 1),
                )

        pend = None
        dmas = [emit_dma(0, chunks=4), emit_dma(1)]
        emit_const_dmas()
        for j in range(0, t - 1, 2):
            if j + 2 < t:
                dmas.append(emit_dma(j + 2))
            num2 = emit_back_act(pend) if pend else None
            cur = new_pair(j)
            emit_front(j, dmas.pop(0), cur, 0)
            if j + 3 < t:
                dmas.append(emit_dma(j + 3))
            emit_front(j + 1, dmas.pop(0), cur, 1)
            if pend:
                emit_back(pend, num2)
            pend = cur
        # single-width tail tile: its back is the only drain work
        s_last = new_pair(t - 1)
        emit_front(t - 1, dmas.pop(0), s_last, 0)
        num2 = emit_back_act(pend)
        emit_back(pend, num2)
        n_last = emit_back_act(s_last, nt=1)
        emit_back(s_last, n_last, nt=1, last=True)

        hist = consts.tile([NB2 * HJ, KV * HJ], f32, tag="hist")
        nc.scalar.copy(out=hist[:], in_=ph[:])
        nc.sync.dma_start(out=out[:, :], in_=hist[:])

    nc.finalize()
    return nc


# ---------------------------------------------------------------- host side

def _prep_core_inputs(logits, labels, core):
    """Build the per-core input dict (fp16, padded, tile-layout xlab)."""
    lo = core * REAL_ROWS_PER_CORE
    hi = lo + REAL_ROWS_PER_CORE
    x = np.full((ROWS_PER_CORE, C), PAD, dtype=np.float16)
    x16 = np.asarray(logits[lo:hi], dtype=np.float16)
    x[:REAL_ROWS_PER_CORE] = x16
    lab = np.asarray(labels[lo:hi]).astype(np.int64)
    xl = np.full(ROWS_PER_CORE, PAD, dtype=np.float16)
    xl[:REAL_ROWS_PER_CORE] = x16[np.arange(REAL_ROWS_PER_CORE), lab]
    return {"x": x, "xlab": xl.reshape(T, P, R)}


def _shared_inputs():
    thr = np.full(NB2, 2.0, dtype=np.float32)
    thr[:NBINS] = np.arange(NBINS, dtype=np.float32) / NBINS
    thr_full = np.broadcast_to(
        thr.astype(np.float16)[None, None, :], (P, 2 * R, NB2)
    )
    return {
        "ident": np.eye(P, dtype=np.float16),
        "thr": thr_full.reshape(P, 2 * R * NB2).copy(),
    }


def _finish(hists):
    """hists: list of [128, 32] PSUM dumps whose diagonal [16,4] blocks are
    cumulative-threshold sums -> (ece, mce)."""
    cum = np.zeros((NBINS + 1, 3), dtype=np.float64)
    for h in hists:
        h = h.astype(np.float64)
        for j in range(HJ):
            blk = h[NB2 * j : NB2 * j + NBINS, KV * j : KV * j + KV]
            cum[:NBINS] += blk[:, [0, 2, 3]]
    per_bin = cum[:NBINS] - cum[1:]  # [15, 3]: sum_conf, sum_acc, count
    sum_conf, sum_acc, counts = per_bin[:, 0], per_bin[:, 1], per_bin[:, 2]
    nonempty = counts > 0
    safe = np.where(nonempty, counts, 1.0)
    gap = np.abs(sum_conf / safe - sum_acc / safe)
    n_total = float(2_000_000)
    ece = np.sum(np.where(nonempty, gap * counts / n_total, 0.0))
    mce = np.max(np.where(nonempty, gap, -np.inf)) if nonempty.any() else 1.0
    return np.float32(ece), np.float32(mce)


_NC_CACHE = {}


def kernel(logits, labels):
    from concourse.bass_utils import run_bass_kernel_spmd

    logits = np.asarray(logits, dtype=np.float32)
    labels = np.asarray(labels)

    if "nc" not in _NC_CACHE:
        _NC_CACHE["nc"] = build_nc()
    nc = _NC_CACHE["nc"]

    shared = _shared_inputs()
    in_maps = [
        {**_prep_core_inputs(logits, labels, core), **shared}
        for core in range(NCORES)
    ]
    res = run_bass_kernel_spmd(nc, in_maps, list(range(NCORES)))
    hists = [res.results[i]["out"] for i in range(NCORES)]
    return _finish(hists)


# revision 45
# speedup vs baseline: 1.0082x; 1.0082x over previous
"""Trainium2 Bass kernel for nn_CalibrationError (ECE/MCE over softmax confidences).

Contract: kernel(logits[N,C] f32, labels[N] int64) -> (ece, mce) f32 scalars,
matching reference.py. Internally shards rows across 8 NeuronCores, computes a
15-bin cumulative (sum_conf, sum_acc, count) histogram on-device per core, and
finishes the tiny ECE/MCE arithmetic on host.

Design (fp16 end-to-end, 4-engine balance, pair-batched back half):
  - Host casts logits to fp16 (halves HBM traffic to ~50MB/core) and gathers
    xlab[i] = x16[i, label[i]] so accuracy is just (xlab == rowmax(x)) -- no
    packed-argmax pass over the full data.
  - Act engine does the one unavoidable full pass, e = exp(x) fp16 (~0.87
    ns/elem), one instruction per tile (each extra Act instr costs ~370ns of
    SBUF-access setup); plus a tiny exp(mx) per tile pair (same Exp table ->
    single ACT_TABLE_LOAD; it is hoisted before the next tiles' big exps so
    it never queues behind them).
  - DVE row max: tensor_tensor max tree 100->50->25->13->7 in fp16 2x mode
    (tensor_reduce has no fast mode, so only the last 7-wide step uses it;
    overlapping slices make odd widths free since max tolerates duplicates).
  - PE row sums: identity matmuls accumulate 4-column partials in PSUM (fp16
    moving = 1 cyc/row).
  - The whole post-sum chain (s fold, clamp, reciprocal_approx_fast,
    conf/acc/ones, 16-bin compare, histogram matmuls) is batched over TILE
    PAIRS: double-width tensors halve the ~300ns/instr DVE fixed costs.
  - vals = [conf, conf, acc, ones]: the duplicated conf gives the 16-bin
    threshold compare a stride-1 fp16 last dim (viewed [p,rr,8,2]) -> DVE 2x.
  - Histogram: 8-row-batched PE matmuls (stationary = contiguous 128-wide ge
    slice -- walrus allows exactly one free dim) into one persistent [128,32]
    PSUM accumulator across all tiles; host sums the diagonal [16,4] blocks
    (bin 15 is a dummy with threshold 2.0 so blocks tile the partitions).
  - Per-pair work is software-pipelined one pair deep (front: DMA/exp/tree/
    row-sums; back: s-chain/ge/histogram) so the in-order DVE/Act/PE queues
    never stall on their own pair's cross-engine chain; x DMA is prefetched
    a tile ahead, all xlab rows arrive in one upfront DMA, the bulk consts
    load behind the first x tiles, and the odd tile runs a single-width back
    as the only drain work (PSUM drained by the then-idle Act engine).
  - Pad rows use x = -1000: exp underflows to exactly 0, so conf = 0 and the
    strict conf > 0/15 compare excludes them from every bin.

Self-contained: hardcodes shapes/sharding; only imports the concourse toolchain.
"""

import sys

if "/opt/trn_rl_repo" not in sys.path:
    sys.path.insert(0, "/opt/trn_rl_repo")

import numpy as np

import concourse.bass as bass
import concourse.bacc as bacc
import concourse.mybir as mybir
from concourse.tile import TileContext
from contextlib import ExitStack

# ---------------------------------------------------------------- constants
P = 128          # SBUF partitions
C = 100          # classes
R = 104          # rows per partition per tile
T = 19           # tiles per core: 9 pairs + one single-width tail tile
NCORES = 8
NBINS = 15
NB2 = 16         # bins padded to 16 (bin 15 is a dummy with threshold 2.0) so
                 # the [16,K] histogram blocks tile the 128 PSUM partitions
KV = 4           # vals lanes: [conf, conf-dup, acc, ones]; the duplicated
                 # conf gives the ge compare a stride-1 last dim (DVE 2x mode)
G = 4            # columns per PE row-sum matmul (C = 25 * G exactly)
HJ = 8           # rows per histogram matmul
ROWS_PER_CORE = P * R * T          # 252_928 (incl. padding)
REAL_ROWS_PER_CORE = 2_000_000 // NCORES  # 250_000
PAD = -1000.0    # pad logit: exp() underflows to exactly 0.0

f16 = mybir.dt.float16
f32 = mybir.dt.float32
i16 = mybir.dt.int16
Alu = mybir.AluOpType
Act = mybir.ActivationFunctionType


def build_nc(p=P, c=C, r=R, t=T):
    """Build the per-core Bass module (SPMD: same program on all cores)."""
    nc = bacc.Bacc()
    r2 = 2 * r

    x = nc.declare_dram_parameter("x", [t * p * r, c], f16, isOutput=False)
    xlab = nc.declare_dram_parameter("xlab", [t, p, r], f16, isOutput=False)
    ident = nc.declare_dram_parameter("ident", [p, p], f16, isOutput=False)
    thr = nc.declare_dram_parameter("thr", [p, r2 * NB2], f16, isOutput=False)
    out = nc.declare_dram_parameter("out", [NB2 * HJ, KV * HJ], f32, isOutput=True)

    xv = x[:, :].rearrange("(t p r) c -> t p (r c)", t=t, p=p, r=r)

    with TileContext(nc) as tc, ExitStack() as ctx:
        consts = ctx.enter_context(tc.tile_pool(name="consts", bufs=1))
        xpool = ctx.enter_context(tc.tile_pool(name="xpool", bufs=3))
        work = ctx.enter_context(tc.tile_pool(name="work", bufs=2))
        pairp = ctx.enter_context(tc.tile_pool(name="pairp", bufs=2))
        psum = ctx.enter_context(tc.tile_pool(name="psum", bufs=4, space="PSUM"))
        psacc = ctx.enter_context(tc.tile_pool(name="psacc", bufs=1, space="PSUM"))

        ident_t = consts.tile([p, p], f16, tag="ident_t")
        nc.sync.dma_start(out=ident_t[:], in_=ident[:, :])
        xlab_t = consts.tile([p, t * r], f16, tag="xlab_t")
        thr_full = consts.tile([p, r2 * NB2], f16, tag="thr_full")

        def emit_const_dmas():
            # emitted after the first x tiles: only needed by the first back
            nc.sync.dma_start(
                out=xlab_t[:].rearrange("p (t r) -> p t r", t=t),
                in_=xlab[:, :, :].rearrange("t p r -> p t r"),
            )
            # thr_full[p, rr, b] = b / 15 (fp16), constant across rr (b
            # fastest so histogram stationary slices are one contiguous dim)
            nc.sync.dma_start(out=thr_full[:], in_=thr[:, :])
        # histogram PSUM accumulator, one group across ALL tiles' matmuls
        ph = psacc.tile([NB2 * HJ, KV * HJ], f32, tag="ph")

        # Engine warmups: absorb the const-tile DMA waits on throwaway ops so
        # first-iteration instructions carry few sync waits (walrus limits
        # the wait-command count per instruction).
        warm = psacc.tile([p, 1], f32, tag="warm")
        nc.tensor.matmul(
            warm[:], lhsT=ident_t[:], rhs=ident_t[:, 0:1], start=True, stop=True
        )
        scr_v = consts.tile([p, 1], f16, tag="scr_v")
        nc.vector.tensor_copy(out=scr_v[:], in_=ident_t[:, 0:1])
        scr_g = consts.tile([p, 1], f16, tag="scr_g")
        nc.gpsimd.tensor_tensor(
            out=scr_g[:], in0=ident_t[:, 0:1], in1=scr_v[:], op=Alu.add
        )
        scr_a = consts.tile([p, 1], f16, tag="scr_a")
        nc.scalar.activation(out=scr_a[:], in_=ident_t[:, 0:1], func=Act.Exp)

        def emit_dma(it, chunks=2):
            xt = xpool.tile([p, r * c], f16, tag="xt")
            h = (r // 2) * c
            if chunks == 4:
                q = h // 2
                nc.sync.dma_start(out=xt[:, 0:q], in_=xv[it][:, 0:q])
                nc.sync.dma_start(out=xt[:, q:h], in_=xv[it][:, q:h])
            else:
                nc.sync.dma_start(out=xt[:, 0:h], in_=xv[it][:, 0:h])
            nc.sync.dma_start(out=xt[:, h:], in_=xv[it][:, h:])
            return xt

        def new_pair(j):
            mx2 = pairp.tile([p, r2], f16, tag="mx2")
            return {"j": j, "mx2": mx2, "pss": []}

        def emit_front(it, xt, pair, half):
            """exp + max tree + PE row-sum matmuls for tile `it` (one half of
            a pair; the row max lands in its half of the pair's mx2)."""
            x3 = xt[:].rearrange("p (r c) -> p r c", r=r)

            # e = exp(x), fp16, one instruction per tile: a second instr
            # costs ~370ns of Act SBUF-access setup and Act is the wall
            et = work.tile([p, r * c], f16, tag="et")
            e3 = et[:].rearrange("p (r c) -> p r c", r=r)
            nc.scalar.activation(out=et[:], in_=xt[:], func=Act.Exp)

            # row max over classes: tensor_tensor tree (fp16 2x mode); odd
            # widths via overlapping slices (duplicates are free for max)
            m50 = work.tile([p, r * 50], f16, tag="m50")
            m50v = m50[:].rearrange("p (r c) -> p r c", r=r)
            nc.vector.tensor_tensor(
                out=m50v, in0=x3[:, :, 0:50], in1=x3[:, :, 50:100], op=Alu.max
            )
            m25 = work.tile([p, r * 25], f16, tag="m25")
            m25v = m25[:].rearrange("p (r c) -> p r c", r=r)
            nc.vector.tensor_tensor(
                out=m25v, in0=m50v[:, :, 0:25], in1=m50v[:, :, 25:50], op=Alu.max
            )
            m13 = work.tile([p, r * 13], f16, tag="m13")
            m13v = m13[:].rearrange("p (r c) -> p r c", r=r)
            nc.vector.tensor_tensor(
                out=m13v, in0=m25v[:, :, 0:13], in1=m25v[:, :, 12:25], op=Alu.max
            )
            m7 = work.tile([p, r * 7], f16, tag="m7")
            m7v = m7[:].rearrange("p (r c) -> p r c", r=r)
            nc.vector.tensor_tensor(
                out=m7v, in0=m13v[:, :, 0:7], in1=m13v[:, :, 6:13], op=Alu.max
            )
            nc.vector.tensor_reduce(
                out=pair["mx2"][:, half * r : (half + 1) * r],
                in_=m7v,
                axis=mybir.AxisListType.X,
                op=Alu.max,
            )

            # row sums on PE: 25 identity matmuls of G=4 columns accumulate
            # s-partials in PSUM; DVE folds the partials in the pair's back.
            pss = psum.tile([p, r * G], f32, tag="pss")
            for k in range(c // G):
                nc.tensor.matmul(
                    pss[:],
                    lhsT=ident_t[:],
                    rhs=e3[:, :, k * G : (k + 1) * G],
                    start=(k == 0),
                    stop=(k == c // G - 1),
                )
            pair["pss"].append(pss)

        def emit_back_act(pair, nt=2):
            """exp(mx) for the pair -- emitted before the next tiles' big exps
            so it doesn't queue behind them on the Act engine."""
            w = nt * r
            num2 = pairp.tile([p, r2], f32, tag="num2")
            nc.scalar.activation(
                out=num2[:, 0:w], in_=pair["mx2"][:, 0:w], func=Act.Exp
            )
            return num2

        def emit_back(pair, num2, nt=2, last=False):
            """Double-width s-chain + vals + ge + histogram matmuls for a tile
            pair (emitted one pair late, so the in-order DVE/PE queues never
            stall on this pair's own cross-engine chain). nt=1 handles the
            de-paired tail tiles at single width."""
            j = pair["j"]
            w = nt * r
            s2 = pairp.tile([p, r2], f32, tag="s2")
            for h in range(nt):
                nc.vector.tensor_reduce(
                    out=s2[:, h * r : (h + 1) * r],
                    in_=pair["pss"][h][:].rearrange("p (r g) -> p r g", r=r),
                    axis=mybir.AxisListType.X,
                    op=Alu.add,
                )
            # conf = exp(mx) * 1/max(s, eps); pad rows have exp(mx) == 0
            nc.vector.tensor_scalar_max(s2[:, 0:w], s2[:, 0:w], 1e-30)
            rs2 = pairp.tile([p, r2], f32, tag="rs2")
            nc.vector.reciprocal_approx_fast(out=rs2[:, 0:w], in_=s2[:, 0:w])

            # vals = [conf, conf, acc, ones] laid out [p, rr, 4] (conf twice,
            # adjacent, so ge's in0 has a stride-1 last dim -> DVE 2x mode)
            vals2 = pairp.tile([p, r2 * KV], f16, tag="vals2")
            v4 = vals2[:].rearrange("p (r k) -> p r k", k=KV)[:, 0:w, :]
            nc.vector.tensor_tensor(
                out=v4[:, :, 0:2],
                in0=num2[:, 0:w].rearrange("p (r one) -> p r one", one=1)
                .broadcast_to((p, w, 2)),
                in1=rs2[:, 0:w].rearrange("p (r one) -> p r one", one=1)
                .broadcast_to((p, w, 2)),
                op=Alu.mult,
            )
            nc.vector.tensor_tensor(
                out=v4[:, :, 2],
                in0=xlab_t[:, j * r : j * r + w],
                in1=pair["mx2"][:, 0:w],
                op=Alu.is_equal,
            )
            nc.gpsimd.memset(v4[:, :, 3], 1.0)

            # ge[p, rr, b] = conf16 > b/15 (strict: pad rows have conf == 0).
            # View both sides as [p, rr, 8, 2]: conf pair (stride 1) vs bin
            # pairs (2j, 2j+1) -> every operand 2-byte with stride-1 last dim.
            ge2 = pairp.tile([p, r2 * NB2], f16, tag="ge2")
            g4 = ge2[:].rearrange("p (r j k) -> p r j k", j=NB2 // 2, k=2)[
                :, 0:w, :, :
            ]
            t4 = thr_full[:].rearrange("p (r j k) -> p r j k", j=NB2 // 2, k=2)[
                :, 0:w, :, :
            ]
            c4 = (
                v4[:, :, 0:2]
                .rearrange("p r (j k) -> p r j k", j=1)
                .broadcast_to((p, w, NB2 // 2, 2))
            )
            nc.vector.tensor_tensor(out=g4, in0=c4, in1=t4, op=Alu.is_gt)

            # histogram: 28 matmuls of 8 rows each into the persistent
            # [128,32] PSUM accumulator; diagonal [16,4] blocks hold the real
            # sums and are block-summed on host after one DMA at kernel end.
            # Stationary = 8-row ge slice (one contiguous 128-elem free dim,
            # walrus requires exactly one); moving = vals slice [8, 4].
            vrb = vals2[:].rearrange("p (r k) -> p r k", k=KV)
            for rb in range(w // HJ):
                nc.tensor.matmul(
                    ph[:],
                    lhsT=ge2[:, rb * HJ * NB2 : (rb + 1) * HJ * NB2],
                    rhs=vrb[:, rb * HJ : (rb + 1) * HJ, :],
                    start=(j == 0 and rb == 0),
                    stop=(last and rb == w // HJ - 1),
                )

        pend = None
        dmas = [emit_dma(0, chunks=4), emit_dma(1)]
        emit_const_dmas()
        for j in range(0, t - 1, 2):
            if j + 2 < t:
                dmas.append(emit_dma(j + 2))
            num2 = emit_back_act(pend) if pend else None
            cur = new_pair(j)
            emit_front(j, dmas.pop(0), cur, 0)
            if j + 3 < t:
                dmas.append(emit_dma(j + 3))
            emit_front(j + 1, dmas.pop(0), cur, 1)
            if pend:
                emit_back(pend, num2)
            pend = cur
        # single-width tail tile: its back is the only drain work
        s_last = new_pair(t - 1)
        emit_front(t - 1, dmas.pop(0), s_last, 0)
        num2 = emit_back_act(pend)
        emit_back(pend, num2)
        n_last = emit_back_act(s_last, nt=1)
        emit_back(s_last, n_last, nt=1, last=True)

        hist = consts.tile([NB2 * HJ, KV * HJ], f32, tag="hist")
        nc.scalar.copy(out=hist[:], in_=ph[:])
        nc.sync.dma_start(out=out[:, :], in_=hist[:])

    nc.finalize()
    return nc


# ---------------------------------------------------------------- host side

def _prep_core_inputs(logits, labels, core):
    """Build the per-core input dict (fp16, padded, tile-layout xlab)."""
    lo = core * REAL_ROWS_PER_CORE
    hi = lo + REAL_ROWS_PER_CORE
    x = np.full((ROWS_PER_CORE, C), PAD, dtype=np.float16)
    x16 = np.asarray(logits[lo:hi], dtype=np.float16)
    x[:REAL_ROWS_PER_CORE] = x16
    lab = np.asarray(labels[lo:hi]).astype(np.int64)
    xl = np.full(ROWS_PER_CORE, PAD, dtype=np.float16)
    xl[:REAL_ROWS_PER_CORE] = x16[np.arange(REAL_ROWS_PER_CORE), lab]
    return {"x": x, "xlab": xl.reshape(T, P, R)}


def _shared_inputs():
    thr = np.full(NB2, 2.0, dtype=np.float32)
    thr[:NBINS] = np.arange(NBINS, dtype=np.float32) / NBINS
    thr_full = np.broadcast_to(
        thr.astype(np.float16)[None, None, :], (P, 2 * R, NB2)
    )
    return {
        "ident": np.eye(P, dtype=np.float16),
        "thr": thr_full.reshape(P, 2 * R * NB2).copy(),
    }


def _finish(hists):
    """hists: list of [128, 32] PSUM dumps whose diagonal [16,4] blocks are
    cumulative-threshold sums -> (ece, mce)."""
    cum = np.zeros((NBINS + 1, 3), dtype=np.float64)
    for h in hists:
        h = h.astype(np.float64)
        for j in range(HJ):
            blk = h[NB2 * j : NB2 * j + NBINS, KV * j : KV * j + KV]
            cum[:NBINS] += blk[:, [0, 2, 3]]
    per_bin = cum[:NBINS] - cum[1:]  # [15, 3]: sum_conf, sum_acc, count
    sum_conf, sum_acc, counts = per_bin[:, 0], per_bin[:, 1], per_bin[:, 2]
    nonempty = counts > 0
    safe = np.where(nonempty, counts, 1.0)
    gap = np.abs(sum_conf / safe - sum_acc / safe)
    n_total = float(2_000_000)
    ece = np.sum(np.where(nonempty, gap * counts / n_total, 0.0))
    mce = np.max(np.where(nonempty, gap, -np.inf)) if nonempty.any() else 1.0
    return np.float32(ece), np.float32(mce)


_NC_CACHE = {}


def kernel(logits, labels):
    from concourse.bass_utils import run_bass_kernel_spmd

    logits = np.asarray(logits, dtype=np.float32)
    labels = np.asarray(labels)

    if "nc" not in _NC_CACHE:
        _NC_CACHE["nc"] = build_nc()
    nc = _NC_CACHE["nc"]

    shared = _shared_inputs()
    in_maps = [
        {**_prep_core_inputs(logits, labels, core), **shared}
        for core in range(NCORES)
    ]
    res = run_bass_kernel_spmd(nc, in_maps, list(range(NCORES)))
    hists = [res.results[i]["out"] for i in range(NCORES)]
    return _finish(hists)
